# revision 1
# baseline (speedup 1.0000x reference)
# Trainium2 Bass kernel for nn_DeformableInception (deformable conv x2 -> concat -> 1x1 conv).
#
# Sharding: data-parallel over batch B=8, one sample per NeuronCore (8 cores).
# Weights replicated. No collectives.
#
# Per-core device pipeline (per sample):
#   - bilinear sampling done as pair-gathers: for each tap/position, the 2x2 corner
#     patch is fetched as two overlapping 512B row-pairs from xT [HW, C] (bf16) in DRAM
#     via SWDGE dma_gather (positions land on partitions).
#   - the 4 corner weights (validity/border-folded, precomputed from the offset maps)
#     are applied as per-partition scalars with tensor_scalar/scalar_tensor_tensor on
#     DVE/GPSIMD, accumulating the 2x2 patch into samp[pos, c].
#   - PE transposes samp -> sampT[c, pos], then the deform conv is PSUM-accumulated
#     matmuls over the 9 taps; the two branch outputs feed the 1x1 fuse conv (also PE).
import sys

sys.path.insert(0, "/opt/trn_rl_repo")

import numpy as np
import ml_dtypes

import concourse.bass as bass
import concourse.mybir as mybir
from concourse.tile import TileContext
from concourse.masks import make_identity
from concourse import bacc
from concourse.bass_utils import run_bass_kernel_spmd

bf16 = ml_dtypes.bfloat16

# problem constants (hardcoded per spec)
B = 8
C = 128
H = W = 64
HW = H * W                 # 4096
COUT = 84
K = 3
PAD = 1
KK = K * K                 # 9
NBR = 2                    # two deformable branches
NTAPS = NBR * KK           # 18
NH = 2                     # process positions in two halves of 2048
HALF = HW // NH            # 2048
NBLK = HALF // 128         # 16 blocks of 128 positions per half
NLISTS = NTAPS * NH        # 36 gather lists, 4096 indices each

P = 128
f32 = mybir.dt.float32
bft = mybir.dt.bfloat16
i16 = mybir.dt.int16

# engine split for the blend: every GP_EVERYth position-block's blend ops go to GPSIMD
# Blend engine split: the Pool (GPSIMD) engine rejects TensorScalarPtr on this
# core version ("Instruction engine check failed (Pool)"), so the whole bilinear
# blend runs on DVE. Keep 0.
import os as _os
GP_EVERY = int(_os.environ.get("KERN_GP_EVERY", "0"))  # 0 = all on DVE
ACT_EVERY = int(_os.environ.get("KERN_ACT_EVERY", "2"))  # every 2nd block's first mul on ACT (HW-validated)
ACC_BUFS = int(_os.environ.get("KERN_ACC_BUFS", "8"))
TPP_BUFS = int(_os.environ.get("KERN_TPP_BUFS", "3"))
GPOOL_BUFS = int(_os.environ.get("KERN_GPOOL_BUFS", "3"))
SAMP_BUFS = int(_os.environ.get("KERN_SAMP_BUFS", "3"))

_CACHE = {}


def _host_precompute(x, dm0, dm1, w0, w1, wf, bfv):
    """Numpy precompute: gather indices + folded bilinear weights, weight repacks."""
    ky = np.repeat(np.arange(K) - PAD, K).astype(np.float32)
    kx = np.tile(np.arange(K) - PAD, K).astype(np.float32)
    base_y = np.arange(H, dtype=np.float32).reshape(1, 1, H, 1)
    base_x = np.arange(W, dtype=np.float32).reshape(1, 1, 1, W)

    idx_all = np.zeros((B, NBR, KK, 2, HW), np.int16)     # [:, :, :, t/b, :]
    w_all = np.zeros((B, NBR, KK, 4, HW), np.float32)     # wtA,wtB,wbA,wbB

    for br, dm in ((0, dm0), (1, dm1)):
        off = dm.reshape(B, KK, 2, H, W)
        py = off[:, :, 0] + base_y + ky.reshape(1, KK, 1, 1)
        px = off[:, :, 1] + base_x + kx.reshape(1, KK, 1, 1)
        y0 = np.floor(py); x0 = np.floor(px)
        wy1 = py - y0; wx1 = px - x0
        wy0 = 1.0 - wy1; wx0 = 1.0 - wx1
        y0i = y0.astype(np.int64); x0i = x0.astype(np.int64)
        xb = np.clip(x0i, 0, W - 2)
        for r, (yi, wy) in enumerate(((y0i, wy0), (y0i + 1, wy1))):
            rowvalid = ((yi >= 0) & (yi < H)).astype(np.float32)
            yc = np.clip(yi, 0, H - 1)
            idx_all[:, br, :, r, :] = (yc * W + xb).reshape(B, KK, HW).astype(np.int16)
            wA = np.zeros_like(wy); wB = np.zeros_like(wy)
            for xi, wx in ((x0i, wx0), (x0i + 1, wx1)):
                colvalid = ((xi >= 0) & (xi < W)).astype(np.float32)
                xc = np.clip(xi, 0, W - 1)
                wc = wy * wx * rowvalid * colvalid
                wA += np.where(xc == xb, wc, 0.0)
                wB += np.where(xc == xb + 1, wc, 0.0)
            w_all[:, br, :, 2 * r + 0, :] = wA.reshape(B, KK, HW)
            w_all[:, br, :, 2 * r + 1, :] = wB.reshape(B, KK, HW)

    # xT [B, HW+1, C] bf16 (pad row so the overlapping pair AP stays in bounds)
    xT = np.transpose(x.reshape(B, C, HW), (0, 2, 1))
    xTp = np.concatenate([xT, np.zeros((B, 1, C), np.float32)], axis=1).astype(bf16)

    # IDX [B, NLISTS, 128, 256] int16: list (br, k, Hh) = top-half ++ bot-half, wrapped
    # (j%16, j//16) and replicated across the 8 gpsimd cores' 16-partition groups.
    seqs = np.zeros((B, NBR, KK, NH, 2, HALF), np.int16)
    for hh in range(NH):
        seqs[:, :, :, hh, 0, :] = idx_all[:, :, :, 0, hh * HALF:(hh + 1) * HALF]
        seqs[:, :, :, hh, 1, :] = idx_all[:, :, :, 1, hh * HALF:(hh + 1) * HALF]
    seqs = seqs.reshape(B, NLISTS, HW)                    # list index L = ((br*KK + k)*NH + hh)
    wrapped = seqs.reshape(B, NLISTS, HW // 16, 16)       # j = col*16 + q
    wrapped = np.transpose(wrapped, (0, 1, 3, 2))         # [B, L, 16, 256]
    IDX = np.broadcast_to(wrapped[:, :, None, :, :], (B, NLISTS, 8, 16, HW // 16))
    IDX = np.ascontiguousarray(IDX.reshape(B, NLISTS, P, HW // 16))

    # WS [B, NLISTS, 128, NBLK*4] f32: per position-block b, 4 corner weights;
    # block b of list (br,k,hh) covers positions hh*HALF + b*128 + p
    wsrc = w_all.reshape(B, NBR, KK, 4, NH, NBLK, P)      # [..., c, hh, b, p]
    WS = np.transpose(wsrc, (0, 1, 2, 4, 6, 5, 3))        # [B, br, k, hh, p, b, c]
    WS = np.ascontiguousarray(WS.reshape(B, NLISTS, P, NBLK * 4), np.float32)

    # W0T [NTAPS, C, COUT] bf16: lhsT per (branch, tap)
    W0T = np.zeros((NTAPS, C, COUT), np.float32)
    for br, w in ((0, w0), (1, w1)):
        for k in range(KK):
            W0T[br * KK + k] = w[:, :, k // K, k % K].T
    W0T = W0T.astype(bf16)

    WFT = np.stack([wf[:, :COUT, 0, 0].T, wf[:, COUT:, 0, 0].T]).astype(bf16)  # [2,84,84]
    BF = bfv.reshape(COUT, 1).astype(np.float32)
    return xTp, IDX, WS, W0T, WFT, BF


def _build_nc():
    nc = bacc.Bacc()
    xT_d = nc.declare_dram_parameter("xT", [HW + 1, C], bft, isOutput=False)
    idx_d = nc.declare_dram_parameter("idx", [NLISTS, P, HW // 16], i16, isOutput=False)
    ws_d = nc.declare_dram_parameter("ws", [NLISTS, P, NBLK * 4], f32, isOutput=False)
    w0_d = nc.declare_dram_parameter("w0t", [NTAPS, C, COUT], bft, isOutput=False)
    wf_d = nc.declare_dram_parameter("wft", [2, COUT, COUT], bft, isOutput=False)
    bf_d = nc.declare_dram_parameter("bfv", [COUT, 1], f32, isOutput=False)
    out_d = nc.declare_dram_parameter("out", [COUT, HW], f32, isOutput=True)

    src_ap = bass.AP(tensor=xT_d, offset=0, ap=[[C, HW], [1, 2 * C]])

    with TileContext(nc) as tc:
        with tc.tile_pool(name="const", bufs=1) as const, \
             tc.tile_pool(name="gp", bufs=GPOOL_BUFS) as gp, \
             tc.tile_pool(name="accp", bufs=ACC_BUFS) as accp, \
             tc.tile_pool(name="sampp", bufs=SAMP_BUFS) as sampp, \
             tc.tile_pool(name="op", bufs=2) as op, \
             tc.tile_pool(name="tpp", bufs=TPP_BUFS, space="PSUM") as tpp, \
             tc.tile_pool(name="bigp", bufs=1, space="PSUM") as bigp:
            ident = const.tile([P, P], bft)
            make_identity(nc, ident[:])
            idx_t = const.tile([P, NLISTS, HW // 16], i16)
            ws_t = const.tile([P, NLISTS, NBLK * 4], f32)
            for L in range(NLISTS):
                nc.sync.dma_start(out=idx_t[:, L, :], in_=idx_d[L])
                nc.sync.dma_start(out=ws_t[:, L, :], in_=ws_d[L])
            w0_t = const.tile([C, NTAPS, COUT], bft)
            for t in range(NTAPS):
                nc.sync.dma_start(out=w0_t[:, t, :], in_=w0_d[t])
            wf_t = const.tile([COUT, 2, COUT], bft)
            nc.sync.dma_start(out=wf_t[:, 0, :], in_=wf_d[0])
            nc.sync.dma_start(out=wf_t[:, 1, :], in_=wf_d[1])
            bf_t = const.tile([COUT, 1], f32)
            nc.sync.dma_start(out=bf_t[:], in_=bf_d[:])

            for hh in range(NH):
                o_sb = []
                for br in range(NBR):
                    out_ps = bigp.tile([COUT, HALF], f32, tag="big")
                    for k in range(KK):
                        L = (br * KK + k) * NH + hh
                        g = gp.tile([P, 2 * NBLK, 2 * C], bft, tag="g")
                        nc.gpsimd.dma_gather(
                            out_ap=g[:], in_ap=src_ap, idxs_ap=idx_t[:, L, :],
                            num_idxs=HW, num_idxs_reg=HW,
                            elem_size=2 * C, elem_step=C, transpose=False,
                            single_packet=False,
                        )
                        sampT = sampp.tile([C, HALF], bft, tag="sampT")
                        for qb in range(NBLK // 4):
                            tp = tpp.tile([C, 512], bft, tag="tp")
                            for j in range(4):
                                b = qb * 4 + j
                                eng = (nc.gpsimd if (GP_EVERY and b % GP_EVERY == GP_EVERY - 1)
                                       else nc.vector)
                                acc = accp.tile([P, C], bft, tag="acc")
                                if ACT_EVERY and b % ACT_EVERY == ACT_EVERY - 1:
                                    # offload the chain's first multiply to ACT
                                    nc.scalar.activation(
                                        out=acc[:], in_=g[:, b, 0:C],
                                        func=mybir.ActivationFunctionType.Identity,
                                        scale=ws_t[:, L, b * 4:b * 4 + 1],
                                    )
                                else:
                                    eng.tensor_scalar(
                                        out=acc[:], in0=g[:, b, 0:C],
                                        scalar1=ws_t[:, L, b * 4:b * 4 + 1], scalar2=None,
                                        op0=mybir.AluOpType.mult,
                                    )
                                for sl, (blk, half0) in enumerate(
                                        ((b, C), (b + NBLK, 0), (b + NBLK, C)), start=1):
                                    eng.scalar_tensor_tensor(
                                        out=acc[:], in0=g[:, blk, half0:half0 + C],
                                        scalar=ws_t[:, L, b * 4 + sl:b * 4 + sl + 1],
                                        in1=acc[:],
                                        op0=mybir.AluOpType.mult, op1=mybir.AluOpType.add,
                                    )
                                nc.tensor.matmul(
                                    out=tp[:, j * P:(j + 1) * P], lhsT=acc[:],
                                    rhs=ident[:], is_transpose=True,
                                    start=(j == 0), stop=(j == 3),
                                )
                            nc.scalar.copy(out=sampT[:, qb * 512:(qb + 1) * 512], in_=tp[:])
                        for cc in range(HALF // 512):
                            nc.tensor.matmul(
                                out=out_ps[:, cc * 512:(cc + 1) * 512],
                                lhsT=w0_t[:, br * KK + k, :],
                                rhs=sampT[:, cc * 512:(cc + 1) * 512],
                                start=(k == 0), stop=(k == KK - 1),
                            )
                    ob = op.tile([COUT, HALF], bft, tag="ob")
                    nc.scalar.copy(out=ob[:], in_=out_ps[:])
                    o_sb.append(ob)
                ps2 = bigp.tile([COUT, HALF], f32, tag="big")
                for cc in range(HALF // 512):
                    sl = slice(cc * 512, (cc + 1) * 512)
                    nc.tensor.matmul(out=ps2[:, sl], lhsT=wf_t[:, 0, :],
                                     rhs=o_sb[0][:, sl], start=True, stop=False)
                    nc.tensor.matmul(out=ps2[:, sl], lhsT=wf_t[:, 1, :],
                                     rhs=o_sb[1][:, sl], start=False, stop=True)
                out_sb = op.tile([COUT, HALF], f32, tag="outsb")
                nc.scalar.activation(
                    out=out_sb[:], in_=ps2[:],
                    func=mybir.ActivationFunctionType.Identity, bias=bf_t[:], scale=1.0,
                )
                nc.sync.dma_start(out=out_d[:, hh * HALF:(hh + 1) * HALF], in_=out_sb[:])
    nc.finalize()
    return nc


def kernel(x, dm0, dm1, w0, w1, wf, bf):
    x = np.asarray(x, np.float32)
    dm0 = np.asarray(dm0, np.float32)
    dm1 = np.asarray(dm1, np.float32)
    w0 = np.asarray(w0, np.float32)
    w1 = np.asarray(w1, np.float32)
    wf = np.asarray(wf, np.float32)
    bfv = np.asarray(bf, np.float32)

    xTp, IDX, WS, W0T, WFT, BF = _host_precompute(x, dm0, dm1, w0, w1, wf, bfv)

    if "nc" not in _CACHE:
        _CACHE["nc"] = _build_nc()
    nc = _CACHE["nc"]

    in_maps = [
        {
            "xT": np.ascontiguousarray(xTp[i]),
            "idx": np.ascontiguousarray(IDX[i]),
            "ws": np.ascontiguousarray(WS[i]),
            "w0t": W0T,
            "wft": WFT,
            "bfv": BF,
        }
        for i in range(B)
    ]
    res = run_bass_kernel_spmd(nc, in_maps, core_ids=list(range(B)),
                               **_CACHE.get("run_kwargs", {}))
    _CACHE["last_results"] = res
    out = np.stack([res.results[i]["out"] for i in range(B)])
    return out.reshape(B, COUT, H, W)



# revision 3
# speedup vs baseline: 1.4114x; 1.4114x over previous
# Trainium2 Bass kernel for nn_DeformableInception (deformable conv x2 -> concat -> 1x1 conv).
#
# Sharding: data-parallel over batch B=8, one sample per NeuronCore (8 cores).
# Weights replicated. No collectives.
#
# Per-core device pipeline (per sample):
#   - x is stored in DRAM as parity-packed 2x2 patches: slot (par, yy, xx) holds
#     image rows (2*yy+par, 2*yy+par+1) x 128ch bf16 (512B). A bilinear 2x2 patch
#     at (yb, xb) is two adjacent slots = ONE contiguous 1KB gather descriptor.
#   - per (tap, half): SWDGE dma_gather fetches 2048 patches (positions land on
#     partitions), g[pos, blk, 512ch] = [v00|v10|v01|v11] per block.
#   - the bilinear blend runs on PE as "diagonal matmuls": for each corner,
#     matmul(out=tp[c, pos], lhsT=g_corner[pos, c], rhs=diag(w_corner)) accumulates
#     the weighted corner into PSUM. The diag tiles (identity * per-position corner
#     weight) are built by cheap 4x-mode tensor_scalar ops on DVE/ACT that depend
#     only on host-precomputed weights, not on the gather.
#   - tp (f32 PSUM) -> sampT (bf16 SBUF), then the deform conv is PSUM-accumulated
#     matmuls over the 9 taps; the two branch outputs feed the 1x1 fuse conv (PE).
import sys

sys.path.insert(0, "/opt/trn_rl_repo")

import numpy as np
import ml_dtypes

import concourse.bass as bass
import concourse.mybir as mybir
from concourse.tile import TileContext
from concourse.masks import make_identity
from concourse import bacc
from concourse.bass_utils import run_bass_kernel_spmd

bf16 = ml_dtypes.bfloat16

# problem constants (hardcoded per spec)
B = 8
C = 128
H = W = 64
HW = H * W                 # 4096
COUT = 84
K = 3
PAD = 1
KK = K * K                 # 9
NBR = 2                    # two deformable branches
NH = 2                     # process positions in two halves of 2048
HALF = HW // NH            # 2048
NBLK = HALF // 128         # 16 blocks of 128 positions per half
NLISTS = NBR * KK * NH     # 36 gather lists, 2048 patch indices each
NSLOT = 2 * 32 * 64        # 4096 parity-packed patch slots

P = 128
f32 = mybir.dt.float32
bft = mybir.dt.bfloat16
i16 = mybir.dt.int16

import os as _os
# every DIAG_ACT_EVERY-th diag build goes to ACT instead of DVE
DIAG_ACT_EVERY = int(_os.environ.get("KERN_DIAG_ACT_EVERY", "4"))
# tp->sampT copy engine: 0=Pool, 1=DVE, 2=ACT (per-copy round robin list)
COPY_ENGS = _os.environ.get("KERN_COPY_ENGS", "0")
GP_BUFS = int(_os.environ.get("KERN_GP_BUFS", "3"))
TPP_BUFS = int(_os.environ.get("KERN_TPP_BUFS", "3"))
SAMP_BUFS = int(_os.environ.get("KERN_SAMP_BUFS", "3"))
DIAG_BUFS = int(_os.environ.get("KERN_DIAG_BUFS", "16"))

_CACHE = {}


def _host_precompute(x, dm0, dm1, w0, w1, wf, bfv):
    """Numpy precompute: patch-slot gather indices + 2D-folded bilinear corner
    weights, parity-packed x, weight repacks."""
    ky = np.repeat(np.arange(K) - PAD, K).astype(np.float32)
    kx = np.tile(np.arange(K) - PAD, K).astype(np.float32)
    base_y = np.arange(H, dtype=np.float32).reshape(1, 1, H, 1)
    base_x = np.arange(W, dtype=np.float32).reshape(1, 1, 1, W)

    idx_all = np.zeros((B, NBR, KK, HW), np.int16)        # patch slot per (tap,pos)
    w_all = np.zeros((B, NBR, KK, 4, HW), np.float32)     # r0c0,r1c0,r0c1,r1c1

    for br, dm in ((0, dm0), (1, dm1)):
        off = dm.reshape(B, KK, 2, H, W)
        py = off[:, :, 0] + base_y + ky.reshape(1, KK, 1, 1)
        px = off[:, :, 1] + base_x + kx.reshape(1, KK, 1, 1)
        y0 = np.floor(py); x0 = np.floor(px)
        wy1 = py - y0; wx1 = px - x0
        wy0 = 1.0 - wy1; wx0 = 1.0 - wx1
        y0i = y0.astype(np.int64); x0i = x0.astype(np.int64)
        yb = np.clip(y0i, 0, H - 2)
        xb = np.clip(x0i, 0, W - 2)
        slot = (yb & 1) * (32 * 64) + (yb >> 1) * 64 + xb
        idx_all[:, br] = slot.reshape(B, KK, HW).astype(np.int16)
        w4 = np.zeros((2, 2) + py.shape, np.float32)      # [rp, cp, B, KK, H, W]
        for r, wy in ((0, wy0), (1, wy1)):
            yi = y0i + r
            rv = ((yi >= 0) & (yi < H)).astype(np.float32)
            rp = np.clip(yi, 0, H - 1) - yb               # 0 or 1
            for c, wx in ((0, wx0), (1, wx1)):
                xi = x0i + c
                cv = ((xi >= 0) & (xi < W)).astype(np.float32)
                cp = np.clip(xi, 0, W - 1) - xb
                contrib = wy * wx * rv * cv
                for rr in (0, 1):
                    for cc in (0, 1):
                        w4[rr, cc] += np.where((rp == rr) & (cp == cc), contrib, 0.0)
        # corner order matches patch byte layout [v00, v10, v01, v11]
        w_all[:, br, :, 0] = w4[0, 0].reshape(B, KK, HW)
        w_all[:, br, :, 1] = w4[1, 0].reshape(B, KK, HW)
        w_all[:, br, :, 2] = w4[0, 1].reshape(B, KK, HW)
        w_all[:, br, :, 3] = w4[1, 1].reshape(B, KK, HW)

    # xPP [B, NSLOT+2, 2C] bf16: slot (par, yy, xx) = rows (2yy+par, 2yy+par+1)
    xhwc = np.transpose(x, (0, 2, 3, 1))                  # [B, H, W, C]
    xPP = np.zeros((B, 2, 32, 64, 2, C), np.float32)
    for par in (0, 1):
        for rp in (0, 1):
            start = par + rp
            rows = xhwc[:, start::2, :, :]                # [B, n, W, C]
            n = min(rows.shape[1], 32)
            xPP[:, par, :n, :, rp, :] = rows[:, :n]
    xPP = xPP.reshape(B, NSLOT, 2 * C)
    xPPp = np.concatenate([xPP, np.zeros((B, 2, 2 * C), np.float32)], axis=1)
    xPPp = xPPp.astype(bf16)

    # IDX [B, 128, NLISTS, HALF//16] int16: list L=(br*KK+k)*NH+hh, wrapped
    # (j%16, j//16), replicated across the 8 gpsimd cores' 16-partition groups,
    # transposed to the device SBUF layout so one contiguous DMA loads it.
    seqs = idx_all.reshape(B, NBR * KK, NH, HALF)
    seqs = seqs.reshape(B, NLISTS, HALF)                  # [B, L, 2048]
    wrapped = seqs.reshape(B, NLISTS, HALF // 16, 16)
    wrapped = np.transpose(wrapped, (0, 1, 3, 2))         # [B, L, 16, 128]
    IDX = np.broadcast_to(wrapped[:, :, None, :, :],
                          (B, NLISTS, 8, 16, HALF // 16))
    IDX = IDX.reshape(B, NLISTS, P, HALF // 16)
    IDX = np.ascontiguousarray(np.transpose(IDX, (0, 2, 1, 3)))  # [B,128,L,128]

    # WS [B, 128, NLISTS, NBLK*4] f32, device layout (partition-major)
    wsrc = w_all.reshape(B, NBR * KK, 4, NH, NBLK, P)     # [..., c4, hh, b, p]
    WS = np.transpose(wsrc, (0, 1, 3, 5, 4, 2))           # [B, t, hh, p, b, c4]
    WS = WS.reshape(B, NLISTS, P, NBLK * 4)
    WS = np.ascontiguousarray(np.transpose(WS, (0, 2, 1, 3)), np.float32)

    # W0T [128, NTAPS*COUT] bf16: lhsT per (branch, tap), device layout
    W0T = np.zeros((NBR * KK, C, COUT), np.float32)
    for br, w in ((0, w0), (1, w1)):
        for k in range(KK):
            W0T[br * KK + k] = w[:, :, k // K, k % K].T
    W0T = np.ascontiguousarray(np.transpose(W0T, (1, 0, 2))).astype(bf16)

    WFT = np.stack([wf[:, :COUT, 0, 0].T, wf[:, COUT:, 0, 0].T]).astype(bf16)
    WFT = np.ascontiguousarray(np.transpose(WFT, (1, 0, 2)))  # [84, 2, 84]
    BF = bfv.reshape(COUT, 1).astype(np.float32)
    return xPPp, IDX, WS, W0T, WFT, BF


def _build_nc():
    nc = bacc.Bacc()
    xpp_d = nc.declare_dram_parameter("xpp", [NSLOT + 2, 2 * C], bft, isOutput=False)
    idx_d = nc.declare_dram_parameter("idx", [P, NLISTS * (HALF // 16)], i16, isOutput=False)
    ws_d = nc.declare_dram_parameter("ws", [P, NLISTS * NBLK * 4], f32, isOutput=False)
    w0_d = nc.declare_dram_parameter("w0t", [C, NBR * KK * COUT], bft, isOutput=False)
    wf_d = nc.declare_dram_parameter("wft", [COUT, 2 * COUT], bft, isOutput=False)
    bf_d = nc.declare_dram_parameter("bfv", [COUT, 1], f32, isOutput=False)
    out_d = nc.declare_dram_parameter("out", [COUT, HW], f32, isOutput=True)

    # patch gather source: elem i = slot i (256 elems), read 512 elems (2 slots)
    src_ap = bass.AP(tensor=xpp_d, offset=0, ap=[[2 * C, NSLOT], [1, 4 * C]])

    copy_engs = [int(t) for t in COPY_ENGS.split(",")]

    with TileContext(nc) as tc:
        with tc.tile_pool(name="const", bufs=1) as const, \
             tc.tile_pool(name="gp", bufs=GP_BUFS) as gp, \
             tc.tile_pool(name="dgp", bufs=DIAG_BUFS) as dgp, \
             tc.tile_pool(name="sampp", bufs=SAMP_BUFS) as sampp, \
             tc.tile_pool(name="op", bufs=2) as op, \
             tc.tile_pool(name="tpp", bufs=TPP_BUFS, space="PSUM") as tpp, \
             tc.tile_pool(name="bigp", bufs=1, space="PSUM") as bigp:
            ident = const.tile([P, P], bft)
            make_identity(nc, ident[:])
            idx_t = const.tile([P, NLISTS, HALF // 16], i16)
            nc.sync.dma_start(out=idx_t[:], in_=idx_d[:])
            ws_t = const.tile([P, NLISTS, NBLK * 4], f32)
            nc.sync.dma_start(out=ws_t[:], in_=ws_d[:])
            w0_t = const.tile([C, NBR * KK, COUT], bft)
            nc.sync.dma_start(out=w0_t[:], in_=w0_d[:])
            wf_t = const.tile([COUT, 2, COUT], bft)
            nc.sync.dma_start(out=wf_t[:], in_=wf_d[:])
            bf_t = const.tile([COUT, 1], f32)
            nc.sync.dma_start(out=bf_t[:], in_=bf_d[:])

            ndiag = 0
            ncopy = 0
            for hh in range(NH):
                o_sb = []
                for br in range(NBR):
                    out_ps = bigp.tile([COUT, HALF], f32, tag="big")
                    for k in range(KK):
                        t = br * KK + k
                        L = t * NH + hh
                        g = gp.tile([P, NBLK, 4 * C], bft, tag="g")
                        nc.gpsimd.dma_gather(
                            out_ap=g[:], in_ap=src_ap, idxs_ap=idx_t[:, L, :],
                            num_idxs=HALF, num_idxs_reg=HALF,
                            elem_size=4 * C, elem_step=2 * C, transpose=False,
                            single_packet=False,
                        )
                        sampT = sampp.tile([C, HALF], bft, tag="sampT")
                        for qb in range(NBLK // 4):
                            tp = tpp.tile([C, 512], f32, tag="tp")
                            for j in range(4):
                                b = qb * 4 + j
                                for c4 in range(4):
                                    diag = dgp.tile([P, P], bft, tag="diag")
                                    sc = ws_t[:, L, b * 4 + c4:b * 4 + c4 + 1]
                                    ndiag += 1
                                    if DIAG_ACT_EVERY and \
                                            ndiag % DIAG_ACT_EVERY == 0:
                                        nc.scalar.activation(
                                            out=diag[:], in_=ident[:],
                                            func=mybir.ActivationFunctionType.Identity,
                                            scale=sc,
                                        )
                                    else:
                                        nc.vector.tensor_scalar(
                                            out=diag[:], in0=ident[:],
                                            scalar1=sc, scalar2=None,
                                            op0=mybir.AluOpType.mult,
                                        )
                                    nc.tensor.matmul(
                                        out=tp[:, j * P:(j + 1) * P],
                                        lhsT=g[:, b, c4 * C:(c4 + 1) * C],
                                        rhs=diag[:],
                                        start=(c4 == 0), stop=(c4 == 3),
                                    )
                            ce = copy_engs[ncopy % len(copy_engs)]
                            ncopy += 1
                            dst = sampT[:, qb * 512:(qb + 1) * 512]
                            if ce == 0:
                                nc.gpsimd.tensor_copy(out=dst, in_=tp[:])
                            elif ce == 1:
                                nc.vector.tensor_copy(out=dst, in_=tp[:])
                            else:
                                nc.scalar.copy(out=dst, in_=tp[:])
                        for cc in range(HALF // 512):
                            nc.tensor.matmul(
                                out=out_ps[:, cc * 512:(cc + 1) * 512],
                                lhsT=w0_t[:, t, :],
                                rhs=sampT[:, cc * 512:(cc + 1) * 512],
                                start=(k == 0), stop=(k == KK - 1),
                            )
                    ob = op.tile([COUT, HALF], bft, tag="ob")
                    nc.scalar.copy(out=ob[:], in_=out_ps[:])
                    o_sb.append(ob)
                ps2 = bigp.tile([COUT, HALF], f32, tag="big")
                for cc in range(HALF // 512):
                    sl = slice(cc * 512, (cc + 1) * 512)
                    nc.tensor.matmul(out=ps2[:, sl], lhsT=wf_t[:, 0, :],
                                     rhs=o_sb[0][:, sl], start=True, stop=False)
                    nc.tensor.matmul(out=ps2[:, sl], lhsT=wf_t[:, 1, :],
                                     rhs=o_sb[1][:, sl], start=False, stop=True)
                out_sb = op.tile([COUT, HALF], f32, tag="outsb")
                nc.scalar.activation(
                    out=out_sb[:], in_=ps2[:],
                    func=mybir.ActivationFunctionType.Identity, bias=bf_t[:], scale=1.0,
                )
                nc.sync.dma_start(out=out_d[:, hh * HALF:(hh + 1) * HALF], in_=out_sb[:])
    nc.finalize()
    return nc


def kernel(x, dm0, dm1, w0, w1, wf, bf):
    x = np.asarray(x, np.float32)
    dm0 = np.asarray(dm0, np.float32)
    dm1 = np.asarray(dm1, np.float32)
    w0 = np.asarray(w0, np.float32)
    w1 = np.asarray(w1, np.float32)
    wf = np.asarray(wf, np.float32)
    bfv = np.asarray(bf, np.float32)

    xPPp, IDX, WS, W0T, WFT, BF = _host_precompute(x, dm0, dm1, w0, w1, wf, bfv)

    if "nc" not in _CACHE:
        _CACHE["nc"] = _build_nc()
    nc = _CACHE["nc"]

    in_maps = [
        {
            "xpp": np.ascontiguousarray(xPPp[i]),
            "idx": np.ascontiguousarray(IDX[i].reshape(P, -1)),
            "ws": np.ascontiguousarray(WS[i].reshape(P, -1)),
            "w0t": W0T.reshape(C, -1),
            "wft": WFT.reshape(COUT, -1),
            "bfv": BF,
        }
        for i in range(B)
    ]
    res = run_bass_kernel_spmd(nc, in_maps, core_ids=list(range(B)),
                               **_CACHE.get("run_kwargs", {}))
    _CACHE["last_results"] = res
    out = np.stack([res.results[i]["out"] for i in range(B)])
    return out.reshape(B, COUT, H, W)


# revision 12
# speedup vs baseline: 1.7406x; 1.2332x over previous
# Trainium2 Bass kernel for nn_DeformableInception (deformable conv x2 -> concat -> 1x1 conv).
#
# Sharding: data-parallel over batch B=8, one sample per NeuronCore (8 cores).
# Weights replicated. No collectives.
#
# Per-core device pipeline (per sample):
#   - x is stored in DRAM as parity-packed 2x2 patches: slot (par, yy, xx) holds
#     image rows (2*yy+par, 2*yy+par+1) x 128ch bf16 (512B). A bilinear 2x2 patch
#     at (yb, xb) is two adjacent slots = ONE contiguous 1KB gather descriptor.
#   - per (tap, half): SWDGE dma_gather fetches 2048 patches (positions land on
#     partitions), g[pos, blk, 512ch] = [v00|v10|v01|v11] per block.
#   - the bilinear blend runs on PE as "diagonal matmuls": for each corner,
#     matmul(out=tp[c, pos], lhsT=g_corner[pos, c], rhs=diag(w_corner)) accumulates
#     the weighted corner into PSUM. The diag tiles (identity * per-position corner
#     weight) are built by cheap 4x-mode tensor_scalar ops on DVE/ACT that depend
#     only on host-precomputed weights, not on the gather.
#   - tp (f32 PSUM) -> sampT (bf16 SBUF), then the deform conv is PSUM-accumulated
#     matmuls over the 9 taps; the two branch outputs feed the 1x1 fuse conv (PE).
import sys

sys.path.insert(0, "/opt/trn_rl_repo")

import numpy as np
import ml_dtypes

import concourse.bass as bass
import concourse.mybir as mybir
from concourse.tile import TileContext
from concourse.masks import make_identity
from concourse import bacc
from concourse.bass_utils import run_bass_kernel_spmd

bf16 = ml_dtypes.bfloat16

# problem constants (hardcoded per spec)
B = 8
C = 128
H = W = 64
HW = H * W                 # 4096
COUT = 84
K = 3
PAD = 1
KK = K * K                 # 9
NBR = 2                    # two deformable branches
NH = 2                     # process positions in two halves of 2048
HALF = HW // NH            # 2048
NBLK = HALF // 128         # 16 blocks of 128 positions per half
NLISTS = NBR * KK * NH     # 36 gather lists, 2048 patch indices each
NSLOT = 2 * 32 * 64        # 4096 parity-packed patch slots

P = 128
f32 = mybir.dt.float32
bft = mybir.dt.bfloat16
i16 = mybir.dt.int16

import os as _os
# diag builds go to ACT when (i * DIAG_ACT_NUM) % DIAG_ACT_DEN rolls under
DIAG_ACT_NUM = int(_os.environ.get("KERN_DIAG_ACT_NUM", "2"))
DIAG_ACT_DEN = int(_os.environ.get("KERN_DIAG_ACT_DEN", "14"))
# tp->sampT copy engine: 1=DVE, 2=ACT (per-copy round robin list)
COPY_ENGS = _os.environ.get("KERN_COPY_ENGS", "2")
GP_BUFS = int(_os.environ.get("KERN_GP_BUFS", "4"))
TPP_BUFS = int(_os.environ.get("KERN_TPP_BUFS", "2"))
SAMP_BUFS = int(_os.environ.get("KERN_SAMP_BUFS", "3"))
DIAG_BUFS = int(_os.environ.get("KERN_DIAG_BUFS", "4"))
DIAG_GRP = int(_os.environ.get("KERN_DIAG_GRP", "8"))  # diags per pool tile

_CACHE = {}


def _host_precompute(x, dm0, dm1, w0, w1, wf, bfv):
    """Numpy precompute: patch-slot gather indices + 2D-folded bilinear corner
    weights, parity-packed x, weight repacks."""
    ky = np.repeat(np.arange(K) - PAD, K).astype(np.float32)
    kx = np.tile(np.arange(K) - PAD, K).astype(np.float32)
    base_y = np.arange(H, dtype=np.float32).reshape(1, 1, H, 1)
    base_x = np.arange(W, dtype=np.float32).reshape(1, 1, 1, W)

    idx_all = np.zeros((B, NBR, KK, HW), np.int16)        # patch slot per (tap,pos)
    w_all = np.zeros((B, NBR, KK, 4, HW), np.float32)     # r0c0,r1c0,r0c1,r1c1

    for br, dm in ((0, dm0), (1, dm1)):
        off = dm.reshape(B, KK, 2, H, W)
        py = off[:, :, 0] + base_y + ky.reshape(1, KK, 1, 1)
        px = off[:, :, 1] + base_x + kx.reshape(1, KK, 1, 1)
        y0 = np.floor(py); x0 = np.floor(px)
        wy1 = py - y0; wx1 = px - x0
        wy0 = 1.0 - wy1; wx0 = 1.0 - wx1
        y0i = y0.astype(np.int64); x0i = x0.astype(np.int64)
        yb = np.clip(y0i, 0, H - 2)
        xb = np.clip(x0i, 0, W - 2)
        slot = (yb & 1) * (32 * 64) + (yb >> 1) * 64 + xb
        idx_all[:, br] = slot.reshape(B, KK, HW).astype(np.int16)
        w4 = np.zeros((2, 2) + py.shape, np.float32)      # [rp, cp, B, KK, H, W]
        for r, wy in ((0, wy0), (1, wy1)):
            yi = y0i + r
            rv = ((yi >= 0) & (yi < H)).astype(np.float32)
            rp = np.clip(yi, 0, H - 1) - yb               # 0 or 1
            for c, wx in ((0, wx0), (1, wx1)):
                xi = x0i + c
                cv = ((xi >= 0) & (xi < W)).astype(np.float32)
                cp = np.clip(xi, 0, W - 1) - xb
                contrib = wy * wx * rv * cv
                for rr in (0, 1):
                    for cc in (0, 1):
                        w4[rr, cc] += np.where((rp == rr) & (cp == cc), contrib, 0.0)
        # corner order matches patch byte layout [v00, v10, v01, v11]
        w_all[:, br, :, 0] = w4[0, 0].reshape(B, KK, HW)
        w_all[:, br, :, 1] = w4[1, 0].reshape(B, KK, HW)
        w_all[:, br, :, 2] = w4[0, 1].reshape(B, KK, HW)
        w_all[:, br, :, 3] = w4[1, 1].reshape(B, KK, HW)

    # xPP [B, NSLOT+2, 2C] bf16: slot (par, yy, xx) = rows (2yy+par, 2yy+par+1)
    xhwc = np.transpose(x, (0, 2, 3, 1))                  # [B, H, W, C]
    xPP = np.zeros((B, 2, 32, 64, 2, C), np.float32)
    for par in (0, 1):
        for rp in (0, 1):
            start = par + rp
            rows = xhwc[:, start::2, :, :]                # [B, n, W, C]
            n = min(rows.shape[1], 32)
            xPP[:, par, :n, :, rp, :] = rows[:, :n]
    xPP = xPP.reshape(B, NSLOT, 2 * C)
    xPPp = np.concatenate([xPP, np.zeros((B, 2, 2 * C), np.float32)], axis=1)
    xPPp = xPPp.astype(bf16)

    # IDX [B, 128, NLISTS, HALF//16] int16: list L=(br*KK+k)*NH+hh, wrapped
    # (j%16, j//16), replicated across the 8 gpsimd cores' 16-partition groups,
    # transposed to the device SBUF layout so one contiguous DMA loads it.
    seqs = idx_all.reshape(B, NBR * KK, NH, HALF)
    seqs = seqs.reshape(B, NLISTS, HALF)                  # [B, L, 2048]
    wrapped = seqs.reshape(B, NLISTS, HALF // 16, 16)
    wrapped = np.transpose(wrapped, (0, 1, 3, 2))         # [B, L, 16, 128]
    IDX = np.broadcast_to(wrapped[:, :, None, :, :],
                          (B, NLISTS, 8, 16, HALF // 16))
    IDX = IDX.reshape(B, NLISTS, P, HALF // 16)
    IDX = np.ascontiguousarray(np.transpose(IDX, (0, 2, 1, 3)))  # [B,128,L,128]

    # WS [B, 128, NLISTS, NBLK*4] f32, device layout (partition-major)
    wsrc = w_all.reshape(B, NBR * KK, 4, NH, NBLK, P)     # [..., c4, hh, b, p]
    WS = np.transpose(wsrc, (0, 1, 3, 5, 4, 2))           # [B, t, hh, p, b, c4]
    WS = WS.reshape(B, NLISTS, P, NBLK * 4)
    WS = np.ascontiguousarray(np.transpose(WS, (0, 2, 1, 3)), np.float32)

    # W0T [128, NTAPS*COUT] bf16: lhsT per (branch, tap), device layout
    W0T = np.zeros((NBR * KK, C, COUT), np.float32)
    for br, w in ((0, w0), (1, w1)):
        for k in range(KK):
            W0T[br * KK + k] = w[:, :, k // K, k % K].T
    W0T = np.ascontiguousarray(np.transpose(W0T, (1, 0, 2))).astype(bf16)

    WFT = np.stack([wf[:, :COUT, 0, 0].T, wf[:, COUT:, 0, 0].T]).astype(bf16)
    WFT = np.ascontiguousarray(np.transpose(WFT, (1, 0, 2)))  # [84, 2, 84]
    BF = bfv.reshape(COUT, 1).astype(np.float32)
    return xPPp, IDX, WS, W0T, WFT, BF


def _build_nc():
    nc = bacc.Bacc()
    xpp_d = nc.declare_dram_parameter("xpp", [NSLOT + 2, 2 * C], bft, isOutput=False)
    idx_d = nc.declare_dram_parameter("idx", [P, NLISTS * (HALF // 16)], i16, isOutput=False)
    ws_d = nc.declare_dram_parameter("ws", [P, NLISTS * NBLK * 4], f32, isOutput=False)
    w0_d = nc.declare_dram_parameter("w0t", [C, NBR * KK * COUT], bft, isOutput=False)
    wf_d = nc.declare_dram_parameter("wft", [COUT, 2 * COUT], bft, isOutput=False)
    bf_d = nc.declare_dram_parameter("bfv", [COUT, 1], f32, isOutput=False)
    out_d = nc.declare_dram_parameter("out", [COUT, HW], bft, isOutput=True)

    # patch gather source: elem i = slot i (256 elems), read 512 elems (2 slots)
    src_ap = bass.AP(tensor=xpp_d, offset=0, ap=[[2 * C, NSLOT], [1, 4 * C]])

    copy_engs = [int(t) for t in COPY_ENGS.split(",")]

    with TileContext(nc) as tc:
        with tc.tile_pool(name="const", bufs=1) as const, \
             tc.tile_pool(name="gp", bufs=GP_BUFS) as gp, \
             tc.tile_pool(name="dgp", bufs=DIAG_BUFS) as dgp, \
             tc.tile_pool(name="sampp", bufs=SAMP_BUFS) as sampp, \
             tc.tile_pool(name="op", bufs=2) as op, \
             tc.tile_pool(name="tpp", bufs=TPP_BUFS, space="PSUM") as tpp, \
             tc.tile_pool(name="bigp", bufs=1, space="PSUM") as bigp:
            ident = const.tile([P, P], bft)
            make_identity(nc, ident[:])
            idx_t = const.tile([P, NLISTS, HALF // 16], i16)
            nc.sync.dma_start(out=idx_t[:], in_=idx_d[:])
            ws_t = const.tile([P, NLISTS, NBLK * 4], f32)
            nc.sync.dma_start(out=ws_t[:], in_=ws_d[:])
            w0_t = const.tile([C, NBR * KK, COUT], bft)
            nc.sync.dma_start(out=w0_t[:], in_=w0_d[:])
            wf_t = const.tile([COUT, 2, COUT], bft)
            nc.sync.dma_start(out=wf_t[:], in_=wf_d[:])
            bf_t = const.tile([COUT, 1], f32)
            nc.sync.dma_start(out=bf_t[:], in_=bf_d[:])

            ndiag = 0
            ncopy = 0
            for hh in range(NH):
                o_sb = []
                for br in range(NBR):
                    out_ps = bigp.tile([COUT, HALF], f32, tag="big")
                    for k in range(KK):
                        t = br * KK + k
                        L = t * NH + hh
                        g = gp.tile([P, NBLK, 4 * C], bft, tag="g")
                        nc.gpsimd.dma_gather(
                            out_ap=g[:], in_ap=src_ap, idxs_ap=idx_t[:, L, :],
                            num_idxs=HALF, num_idxs_reg=HALF,
                            elem_size=4 * C, elem_step=2 * C, transpose=False,
                            single_packet=False,
                        )
                        sampT = sampp.tile([C, HALF], bft, tag="sampT")
                        for qh in range(2):
                            tp = tpp.tile([C, HALF // 2], f32, tag="tp")
                            dgrp = None
                            for jb in range(NBLK // 2):
                                b = qh * (NBLK // 2) + jb
                                for c4 in range(4):
                                    gi = ndiag % DIAG_GRP
                                    if gi == 0:
                                        dgrp = dgp.tile([P, DIAG_GRP, P], bft,
                                                        tag="diag")
                                    diag = dgrp[:, gi, :]
                                    sc = ws_t[:, L, b * 4 + c4:b * 4 + c4 + 1]
                                    on_act = (ndiag * DIAG_ACT_NUM) \
                                        % DIAG_ACT_DEN < DIAG_ACT_NUM
                                    ndiag += 1
                                    if on_act:
                                        nc.scalar.activation(
                                            out=diag, in_=ident[:],
                                            func=mybir.ActivationFunctionType.Identity,
                                            scale=sc,
                                        )
                                    else:
                                        nc.vector.tensor_scalar(
                                            out=diag, in0=ident[:],
                                            scalar1=sc, scalar2=None,
                                            op0=mybir.AluOpType.mult,
                                        )
                                    nc.tensor.matmul(
                                        out=tp[:, jb * P:(jb + 1) * P],
                                        lhsT=g[:, b, c4 * C:(c4 + 1) * C],
                                        rhs=diag,
                                        start=(c4 == 0), stop=(c4 == 3),
                                    )
                            ce = copy_engs[ncopy % len(copy_engs)]
                            ncopy += 1
                            dst = sampT[:, qh * (HALF // 2):(qh + 1) * (HALF // 2)]
                            if ce == 1:
                                nc.vector.tensor_copy(out=dst, in_=tp[:])
                            else:
                                nc.scalar.copy(out=dst, in_=tp[:])
                        for cc in range(HALF // 512):
                            nc.tensor.matmul(
                                out=out_ps[:, cc * 512:(cc + 1) * 512],
                                lhsT=w0_t[:, t, :],
                                rhs=sampT[:, cc * 512:(cc + 1) * 512],
                                start=(k == 0), stop=(k == KK - 1),
                            )
                    ob = op.tile([COUT, HALF], bft, tag="ob")
                    nc.scalar.copy(out=ob[:], in_=out_ps[:])
                    o_sb.append(ob)
                ps2 = bigp.tile([COUT, HALF], f32, tag="big")
                for cc in range(HALF // 512):
                    sl = slice(cc * 512, (cc + 1) * 512)
                    nc.tensor.matmul(out=ps2[:, sl], lhsT=wf_t[:, 0, :],
                                     rhs=o_sb[0][:, sl], start=True, stop=False)
                    nc.tensor.matmul(out=ps2[:, sl], lhsT=wf_t[:, 1, :],
                                     rhs=o_sb[1][:, sl], start=False, stop=True)
                out_sb = op.tile([COUT, HALF], bft, tag="outsb")
                nc.scalar.activation(
                    out=out_sb[:], in_=ps2[:],
                    func=mybir.ActivationFunctionType.Identity, bias=bf_t[:], scale=1.0,
                )
                nc.sync.dma_start(out=out_d[:, hh * HALF:(hh + 1) * HALF], in_=out_sb[:])
    nc.finalize()
    return nc


def kernel(x, dm0, dm1, w0, w1, wf, bf):
    x = np.asarray(x, np.float32)
    dm0 = np.asarray(dm0, np.float32)
    dm1 = np.asarray(dm1, np.float32)
    w0 = np.asarray(w0, np.float32)
    w1 = np.asarray(w1, np.float32)
    wf = np.asarray(wf, np.float32)
    bfv = np.asarray(bf, np.float32)

    xPPp, IDX, WS, W0T, WFT, BF = _host_precompute(x, dm0, dm1, w0, w1, wf, bfv)

    if "nc" not in _CACHE:
        _CACHE["nc"] = _build_nc()
    nc = _CACHE["nc"]

    in_maps = [
        {
            "xpp": np.ascontiguousarray(xPPp[i]),
            "idx": np.ascontiguousarray(IDX[i].reshape(P, -1)),
            "ws": np.ascontiguousarray(WS[i].reshape(P, -1)),
            "w0t": W0T.reshape(C, -1),
            "wft": WFT.reshape(COUT, -1),
            "bfv": BF,
        }
        for i in range(B)
    ]
    res = run_bass_kernel_spmd(nc, in_maps, core_ids=list(range(B)),
                               **_CACHE.get("run_kwargs", {}))
    _CACHE["last_results"] = res
    out = np.stack([np.asarray(res.results[i]["out"], np.float32)
                    for i in range(B)])
    return out.reshape(B, COUT, H, W)


# revision 22
# speedup vs baseline: 1.7848x; 1.0254x over previous
# Trainium2 Bass kernel for nn_DeformableInception (deformable conv x2 -> concat -> 1x1 conv).
#
# Sharding: data-parallel over batch B=8, one sample per NeuronCore (8 cores).
# Weights replicated. No collectives.
#
# Per-core device pipeline (per sample):
#   - x is stored in DRAM as parity-packed 2x2 patches: slot (par, yy, xx) holds
#     image rows (2*yy+par, 2*yy+par+1) x 128ch bf16 (512B). A bilinear 2x2 patch
#     at (yb, xb) is two adjacent slots = ONE contiguous 1KB gather descriptor.
#   - per (tap, half): SWDGE dma_gather fetches 2048 patches (positions land on
#     partitions), g[pos, blk, 512ch] = [v00|v10|v01|v11] per block.
#   - the bilinear blend runs on PE as "diagonal matmuls": for each corner,
#     matmul(out=tp[c, pos], lhsT=g_corner[pos, c], rhs=diag(w_corner)) accumulates
#     the weighted corner into PSUM. The diag tiles (identity * per-position corner
#     weight) are built by cheap 4x-mode tensor_scalar ops on DVE/ACT that depend
#     only on host-precomputed weights, not on the gather.
#   - tp (f32 PSUM) -> sampT (bf16 SBUF), then the deform conv is PSUM-accumulated
#     matmuls over the 9 taps; the two branch outputs feed the 1x1 fuse conv (PE).
import sys

sys.path.insert(0, "/opt/trn_rl_repo")

import numpy as np
import ml_dtypes

import concourse.bass as bass
import concourse.mybir as mybir
from concourse.tile import TileContext
from concourse.masks import make_identity
from concourse import bacc
from concourse.bass_utils import run_bass_kernel_spmd

bf16 = ml_dtypes.bfloat16

# problem constants (hardcoded per spec)
B = 8
C = 128
H = W = 64
HW = H * W                 # 4096
COUT = 84
K = 3
PAD = 1
KK = K * K                 # 9
NBR = 2                    # two deformable branches
NH = 2                     # process positions in two halves of 2048
HALF = HW // NH            # 2048
NBLK = HALF // 128         # 16 blocks of 128 positions per half
NLISTS = NBR * KK * NH     # 36 gather lists, 2048 patch indices each
NSLOT = 2 * 32 * 64        # 4096 parity-packed patch slots

P = 128
f32 = mybir.dt.float32
bft = mybir.dt.bfloat16
i16 = mybir.dt.int16

import os as _os
# diag builds go to ACT when (i * DIAG_ACT_NUM) % DIAG_ACT_DEN rolls under
DIAG_ACT_NUM = int(_os.environ.get("KERN_DIAG_ACT_NUM", "2"))
DIAG_ACT_DEN = int(_os.environ.get("KERN_DIAG_ACT_DEN", "14"))
# tp->sampT copy engine: 1=DVE, 2=ACT (per-copy round robin list)
COPY_ENGS = _os.environ.get("KERN_COPY_ENGS", "2")
GP_BUFS = int(_os.environ.get("KERN_GP_BUFS", "4"))
TPP_BUFS = int(_os.environ.get("KERN_TPP_BUFS", "2"))
SAMP_BUFS = int(_os.environ.get("KERN_SAMP_BUFS", "3"))
DIAG_BUFS = int(_os.environ.get("KERN_DIAG_BUFS", "4"))
DIAG_GRP = int(_os.environ.get("KERN_DIAG_GRP", "8"))  # diags per pool tile

_CACHE = {}


def _host_precompute(x, dm0, dm1, w0, w1, wf, bfv):
    """Numpy precompute: patch-slot gather indices + 2D-folded bilinear corner
    weights, parity-packed x, weight repacks."""
    ky = np.repeat(np.arange(K) - PAD, K).astype(np.float32)
    kx = np.tile(np.arange(K) - PAD, K).astype(np.float32)
    base_y = np.arange(H, dtype=np.float32).reshape(1, 1, H, 1)
    base_x = np.arange(W, dtype=np.float32).reshape(1, 1, 1, W)

    idx_all = np.zeros((B, NBR, KK, HW), np.int16)        # patch slot per (tap,pos)
    w_all = np.zeros((B, NBR, KK, 4, HW), np.float32)     # r0c0,r1c0,r0c1,r1c1

    for br, dm in ((0, dm0), (1, dm1)):
        off = dm.reshape(B, KK, 2, H, W)
        py = off[:, :, 0] + base_y + ky.reshape(1, KK, 1, 1)
        px = off[:, :, 1] + base_x + kx.reshape(1, KK, 1, 1)
        y0 = np.floor(py); x0 = np.floor(px)
        wy1 = py - y0; wx1 = px - x0
        wy0 = 1.0 - wy1; wx0 = 1.0 - wx1
        y0i = y0.astype(np.int64); x0i = x0.astype(np.int64)
        yb = np.clip(y0i, 0, H - 2)
        xb = np.clip(x0i, 0, W - 2)
        slot = (yb & 1) * (32 * 64) + (yb >> 1) * 64 + xb
        idx_all[:, br] = slot.reshape(B, KK, HW).astype(np.int16)
        w4 = np.zeros((2, 2) + py.shape, np.float32)      # [rp, cp, B, KK, H, W]
        for r, wy in ((0, wy0), (1, wy1)):
            yi = y0i + r
            rv = ((yi >= 0) & (yi < H)).astype(np.float32)
            rp = np.clip(yi, 0, H - 1) - yb               # 0 or 1
            for c, wx in ((0, wx0), (1, wx1)):
                xi = x0i + c
                cv = ((xi >= 0) & (xi < W)).astype(np.float32)
                cp = np.clip(xi, 0, W - 1) - xb
                contrib = wy * wx * rv * cv
                for rr in (0, 1):
                    for cc in (0, 1):
                        w4[rr, cc] += np.where((rp == rr) & (cp == cc), contrib, 0.0)
        # corner order matches patch byte layout [v00, v10, v01, v11]
        w_all[:, br, :, 0] = w4[0, 0].reshape(B, KK, HW)
        w_all[:, br, :, 1] = w4[1, 0].reshape(B, KK, HW)
        w_all[:, br, :, 2] = w4[0, 1].reshape(B, KK, HW)
        w_all[:, br, :, 3] = w4[1, 1].reshape(B, KK, HW)

    # xPP [B, NSLOT+2, 2C] bf16: slot (par, yy, xx) = rows (2yy+par, 2yy+par+1)
    xhwc = np.transpose(x, (0, 2, 3, 1))                  # [B, H, W, C]
    xPP = np.zeros((B, 2, 32, 64, 2, C), np.float32)
    for par in (0, 1):
        for rp in (0, 1):
            start = par + rp
            rows = xhwc[:, start::2, :, :]                # [B, n, W, C]
            n = min(rows.shape[1], 32)
            xPP[:, par, :n, :, rp, :] = rows[:, :n]
    xPP = xPP.reshape(B, NSLOT, 2 * C)
    xPPp = np.concatenate([xPP, np.zeros((B, 2, 2 * C), np.float32)], axis=1)
    xPPp = xPPp.astype(bf16)

    # IDX [B, 128, NLISTS, HALF//16] int16: list L=(hh*NBR+br)*KK+k (consumption
    # order), wrapped (j%16, j//16), replicated across the 8 gpsimd cores'
    # 16-partition groups, transposed so one contiguous DMA loads it.
    seqs = idx_all.reshape(B, NBR * KK, NH, HALF)
    seqs = np.transpose(seqs, (0, 2, 1, 3))               # [B, hh, t, HALF]
    seqs = seqs.reshape(B, NLISTS, HALF)                  # [B, L, 2048]
    wrapped = seqs.reshape(B, NLISTS, HALF // 16, 16)
    wrapped = np.transpose(wrapped, (0, 1, 3, 2))         # [B, L, 16, 128]
    IDX = np.broadcast_to(wrapped[:, :, None, :, :],
                          (B, NLISTS, 8, 16, HALF // 16))
    IDX = IDX.reshape(B, NLISTS, P, HALF // 16)
    IDX = np.ascontiguousarray(np.transpose(IDX, (0, 2, 1, 3)))  # [B,128,L,128]

    # WS [B, 128, NLISTS, NBLK*4] f32, device layout (partition-major)
    wsrc = w_all.reshape(B, NBR * KK, 4, NH, NBLK, P)     # [..., c4, hh, b, p]
    WS = np.transpose(wsrc, (0, 3, 1, 5, 4, 2))           # [B, hh, t, p, b, c4]
    WS = WS.reshape(B, NLISTS, P, NBLK * 4)
    WS = np.ascontiguousarray(np.transpose(WS, (0, 2, 1, 3)), np.float32)

    # W0T [128, NTAPS*COUT] bf16: lhsT per (branch, tap) with the 1x1 fuse conv
    # folded in (W''_k = W_k @ Wf_br^T), device layout
    WFT = [wf[:, :COUT, 0, 0].T, wf[:, COUT:, 0, 0].T]    # [84in, 84out] per br
    W0T = np.zeros((NBR * KK, C, COUT), np.float32)
    for br, w in ((0, w0), (1, w1)):
        for k in range(KK):
            W0T[br * KK + k] = w[:, :, k // K, k % K].T @ WFT[br]
    W0T = np.ascontiguousarray(np.transpose(W0T, (1, 0, 2))).astype(bf16)

    BF = bfv.reshape(COUT, 1).astype(np.float32)
    return xPPp, IDX, WS, W0T, BF


def _build_nc():
    nc = bacc.Bacc()
    xpp_d = nc.declare_dram_parameter("xpp", [NSLOT + 2, 2 * C], bft, isOutput=False)
    idx_d = nc.declare_dram_parameter("idx", [P, NLISTS * (HALF // 16)], i16, isOutput=False)
    ws_d = nc.declare_dram_parameter("ws", [P, NLISTS * NBLK * 4], f32, isOutput=False)
    w0_d = nc.declare_dram_parameter("w0t", [C, NBR * KK * COUT], bft, isOutput=False)
    bf_d = nc.declare_dram_parameter("bfv", [COUT, 1], f32, isOutput=False)
    out_d = nc.declare_dram_parameter("out", [COUT, HW], bft, isOutput=True)

    # patch gather source: elem i = slot i (256 elems), read 512 elems (2 slots)
    src_ap = bass.AP(tensor=xpp_d, offset=0, ap=[[2 * C, NSLOT], [1, 4 * C]])

    copy_engs = [int(t) for t in COPY_ENGS.split(",")]

    with TileContext(nc) as tc:
        with tc.tile_pool(name="const", bufs=1) as const, \
             tc.tile_pool(name="gp", bufs=GP_BUFS) as gp, \
             tc.tile_pool(name="dgp", bufs=DIAG_BUFS) as dgp, \
             tc.tile_pool(name="sampp", bufs=SAMP_BUFS) as sampp, \
             tc.tile_pool(name="op", bufs=2) as op, \
             tc.tile_pool(name="tpp", bufs=TPP_BUFS, space="PSUM") as tpp, \
             tc.tile_pool(name="bigp", bufs=1, space="PSUM") as bigp:
            ident = const.tile([P, P], bft)
            make_identity(nc, ident[:])
            # split the idx/ws loads so the first gathers launch ASAP
            NIH = 2 * (HALF // 16)
            NWH = 2 * (NBLK * 4)
            idx_t = const.tile([P, NLISTS, HALF // 16], i16)
            nc.sync.dma_start(out=idx_t[:, 0:2, :], in_=idx_d[:, 0:NIH])
            ws_t = const.tile([P, NLISTS, NBLK * 4], f32)
            nc.sync.dma_start(out=ws_t[:, 0:2, :], in_=ws_d[:, 0:NWH])
            nc.sync.dma_start(out=idx_t[:, 2:NLISTS, :],
                              in_=idx_d[:, NIH:])
            nc.sync.dma_start(out=ws_t[:, 2:NLISTS, :], in_=ws_d[:, NWH:])
            w0_t = const.tile([C, NBR * KK, COUT], bft)
            nc.sync.dma_start(out=w0_t[:], in_=w0_d[:])
            bf_t = const.tile([COUT, 1], f32)
            nc.sync.dma_start(out=bf_t[:], in_=bf_d[:])

            ndiag = 0
            ncopy = 0
            for hh in range(NH):
                out_ps = bigp.tile([COUT, HALF], f32, tag="big")
                for br in range(NBR):
                    for k in range(KK):
                        t = br * KK + k
                        L = (hh * NBR + br) * KK + k
                        g = gp.tile([P, NBLK, 4 * C], bft, tag="g")
                        nc.gpsimd.dma_gather(
                            out_ap=g[:], in_ap=src_ap, idxs_ap=idx_t[:, L, :],
                            num_idxs=HALF, num_idxs_reg=HALF,
                            elem_size=4 * C, elem_step=2 * C, transpose=False,
                            single_packet=False,
                        )
                        sampT = sampp.tile([C, HALF], bft, tag="sampT")
                        for qh in range(2):
                            tp = tpp.tile([C, HALF // 2], f32, tag="tp")
                            dgrp = None
                            for jb in range(NBLK // 2):
                                b = qh * (NBLK // 2) + jb
                                for c4 in range(4):
                                    gi = ndiag % DIAG_GRP
                                    if gi == 0:
                                        dgrp = dgp.tile([P, DIAG_GRP, P], bft,
                                                        tag="diag")
                                    diag = dgrp[:, gi, :]
                                    sc = ws_t[:, L, b * 4 + c4:b * 4 + c4 + 1]
                                    on_act = (ndiag * DIAG_ACT_NUM) \
                                        % DIAG_ACT_DEN < DIAG_ACT_NUM
                                    ndiag += 1
                                    if on_act:
                                        nc.scalar.activation(
                                            out=diag, in_=ident[:],
                                            func=mybir.ActivationFunctionType.Identity,
                                            scale=sc,
                                        )
                                    else:
                                        nc.vector.tensor_scalar(
                                            out=diag, in0=ident[:],
                                            scalar1=sc, scalar2=None,
                                            op0=mybir.AluOpType.mult,
                                        )
                                    nc.tensor.matmul(
                                        out=tp[:, jb * P:(jb + 1) * P],
                                        lhsT=g[:, b, c4 * C:(c4 + 1) * C],
                                        rhs=diag,
                                        start=(c4 == 0), stop=(c4 == 3),
                                    )
                            ce = copy_engs[ncopy % len(copy_engs)]
                            ncopy += 1
                            dst = sampT[:, qh * (HALF // 2):(qh + 1) * (HALF // 2)]
                            if ce == 1:
                                nc.vector.tensor_copy(out=dst, in_=tp[:])
                            else:
                                nc.scalar.copy(out=dst, in_=tp[:])
                        for cc in range(HALF // 512):
                            nc.tensor.matmul(
                                out=out_ps[:, cc * 512:(cc + 1) * 512],
                                lhsT=w0_t[:, t, :],
                                rhs=sampT[:, cc * 512:(cc + 1) * 512],
                                start=(br == 0 and k == 0),
                                stop=(br == NBR - 1 and k == KK - 1),
                            )
                out_sb = op.tile([COUT, HALF], bft, tag="outsb")
                nc.scalar.activation(
                    out=out_sb[:], in_=out_ps[:],
                    func=mybir.ActivationFunctionType.Identity, bias=bf_t[:], scale=1.0,
                )
                nc.sync.dma_start(out=out_d[:, hh * HALF:(hh + 1) * HALF], in_=out_sb[:])
    nc.finalize()
    return nc


def kernel(x, dm0, dm1, w0, w1, wf, bf):
    x = np.asarray(x, np.float32)
    dm0 = np.asarray(dm0, np.float32)
    dm1 = np.asarray(dm1, np.float32)
    w0 = np.asarray(w0, np.float32)
    w1 = np.asarray(w1, np.float32)
    wf = np.asarray(wf, np.float32)
    bfv = np.asarray(bf, np.float32)

    xPPp, IDX, WS, W0T, BF = _host_precompute(x, dm0, dm1, w0, w1, wf, bfv)

    if "nc" not in _CACHE:
        _CACHE["nc"] = _build_nc()
    nc = _CACHE["nc"]

    in_maps = [
        {
            "xpp": np.ascontiguousarray(xPPp[i]),
            "idx": np.ascontiguousarray(IDX[i].reshape(P, -1)),
            "ws": np.ascontiguousarray(WS[i].reshape(P, -1)),
            "w0t": W0T.reshape(C, -1),
            "bfv": BF,
        }
        for i in range(B)
    ]
    res = run_bass_kernel_spmd(nc, in_maps, core_ids=list(range(B)),
                               **_CACHE.get("run_kwargs", {}))
    _CACHE["last_results"] = res
    out = np.stack([np.asarray(res.results[i]["out"], np.float32)
                    for i in range(B)])
    return out.reshape(B, COUT, H, W)
